# revision 42
# baseline (speedup 1.0000x reference)
"""Trainium2 Bass kernel for nn_EnhancedFusionGate (kNN fusion gate).

Data-parallel over points: each of the 8 cores handles 2048 of the 16384
rows. Phase A finds exact 16-NN per point via a grouped-max pruning pass
(PE fp32r distance matmul + DVE max8 machinery) and computes neighbor
feature means. The 3x3 eigendecompositions run on host CPU (bit-matching
the reference LAPACK path). Phase B runs all MLP gates and the final mix.
"""

import numpy as np

import concourse.bacc as bacc
import concourse.bass as bass
import concourse.mybir as mybir
from concourse.tile import TileContext
from concourse.bass_utils import run_bass_kernel_spmd
from concourse.masks import make_identity

N = 16384
C = 96
N_CORES = 8
NQ = N // N_CORES            # 2048 queries per core
QT = NQ // 128               # 16 query tiles per core
G = 16                       # candidates per pruning group
NG = N // G                  # 1024 groups
SEL_G = 24                   # groups gathered per query (3 max8 rounds)
SC = SEL_G * G               # 384 exact candidates per query
CAND_W = 5                   # x, y, z, sq, idx per candidate
PG_W = G * CAND_W            # 80 floats per group row
K_OUT = 16                   # neighbors returned
NJB = 8                      # phase-B dma_gather blocks per core
QPB = NQ // NJB              # queries per gather block (256)
NEG_BIG = float(np.float32(-3.0e38))

f32 = mybir.dt.float32
f32r = mybir.dt.float32r
i32 = mybir.dt.int32
u32 = mybir.dt.uint32

AF = mybir.ActivationFunctionType
ALU = mybir.AluOpType


def build_phase_a():
    nc = bacc.Bacc("TRN2", target_bir_lowering=False, debug=False,
                   num_devices=N_CORES)
    c5 = nc.dram_tensor("c5", [5, N], f32, kind="ExternalInput").ap()
    q5 = nc.dram_tensor("q5", [5, NQ], f32, kind="ExternalInput").ap()
    qpt = nc.dram_tensor("qpt", [NQ, 4], f32, kind="ExternalInput").ap()
    pg = nc.dram_tensor("pg", [NG, PG_W], f32, kind="ExternalInput").ap()

    pos_out = nc.dram_tensor("pos_out", [NQ, K_OUT], i32,
                             kind="ExternalOutput").ap()
    gidx_out = nc.dram_tensor("gidx_out", [NQ, SEL_G], i32,
                              kind="ExternalOutput").ap()

    bf16 = mybir.dt.bfloat16
    # chunks whose seg-max runs via an ACT bf16 cast-copy instead of a
    # direct DVE read from PSUM (balances the two engines)
    N_CH = N // 512
    act_chunks = set(range(0, N_CH, 2))

    with TileContext(nc) as tc:
        with (
            tc.tile_pool(name="cpool", bufs=1) as cpool,
            tc.tile_pool(name="qpool", bufs=2) as qpool,
            tc.tile_pool(name="gmpool", bufs=2) as gmpool,
            tc.tile_pool(name="selpool", bufs=2) as selpool,
            tc.tile_pool(name="candpool", bufs=2) as candpool,
            tc.tile_pool(name="bfpool", bufs=3) as bfpool,
            tc.tile_pool(name="psum", bufs=8, space="PSUM") as psum,
        ):
            c5s = cpool.tile([5, N], f32r, tag="c5s")
            nc.sync.dma_start(out=c5s[:], in_=c5[:].bitcast(f32r))
            q5s = cpool.tile([5, NQ], f32r, tag="q5s")
            nc.sync.dma_start(out=q5s[:], in_=q5[:].bitcast(f32r))

            for t in range(QT):
                qs = slice(t * 128, (t + 1) * 128)
                # per-query point data (x, y, z, sq) for exact recompute
                qp = qpool.tile([128, 4], f32, tag="qp")
                nc.sync.dma_start(out=qp[:], in_=qpt[qs, :])

                # ---- pruning pass: group maxes of -d2 over all candidates
                gm = gmpool.tile([128, NG], f32, tag="gm")
                for ch in range(N_CH):
                    pt = psum.tile([128, 512], f32, tag="negd2")
                    nc.tensor.matmul(
                        pt[:],
                        q5s[:, qs],
                        c5s[:, ch * 512:(ch + 1) * 512],
                        start=True, stop=True,
                    )
                    if ch in act_chunks:
                        bt = bfpool.tile([128, 512], bf16, tag="bt")
                        nc.scalar.copy(bt[:], pt[:])
                        red_in = bt[:].rearrange("p (g k) -> p g k", k=G)
                    else:
                        red_in = pt[:].rearrange("p (g k) -> p g k", k=G)
                    nc.vector.tensor_reduce(
                        out=gm[:, ch * 32:(ch + 1) * 32],
                        in_=red_in,
                        axis=mybir.AxisListType.X,
                        op=ALU.max,
                    )

                # ---- pick top SEL_G groups by group max (3 rounds of max8)
                gidx = selpool.tile([128, SEL_G], u32, tag="gidx")
                gmw = gm
                for r in range(SEL_G // 8):
                    gv = selpool.tile([128, 8], f32, tag=f"gv{r}")
                    nc.vector.max(out=gv[:], in_=gmw[:])
                    nc.vector.max_index(
                        out=gidx[:, r * 8:(r + 1) * 8], in_max=gv[:],
                        in_values=gmw[:],
                    )
                    if r < SEL_G // 8 - 1:
                        gmn = gmpool.tile([128, NG], f32, tag=f"gmr{r}")
                        nc.vector.match_replace(
                            out=gmn[:], in_to_replace=gv[:], in_values=gmw[:],
                            imm_value=NEG_BIG,
                        )
                        gmw = gmn
                gidx_i = selpool.tile([128, SEL_G], i32, tag="gidx_i")
                nc.vector.tensor_copy(gidx_i[:], gidx[:])
                nc.sync.dma_start(out=gidx_out[qs, :], in_=gidx_i[:])

                # ---- gather the SEL_G groups' candidate data from DRAM
                # (hardware indirect DMA honors one offset per partition,
                # so one DMA per group slot)
                cand = candpool.tile([128, SEL_G * PG_W], f32, tag="cand")
                for s in range(SEL_G):
                    nc.gpsimd.indirect_dma_start(
                        out=cand[:, s * PG_W:(s + 1) * PG_W],
                        out_offset=None,
                        in_=pg[:, :],
                        in_offset=bass.IndirectOffsetOnAxis(
                            ap=gidx[:, s:s + 1], axis=0),
                    )
                cpl = cand[:].rearrange("p (s v) -> p s v", v=CAND_W)
                xc, yc = cpl[:, :, 0], cpl[:, :, 1]
                zc, sqc = cpl[:, :, 2], cpl[:, :, 3]

                # ---- exact -d2 recompute, mirroring the reference fp32 ops
                t1 = candpool.tile([128, SC], f32, tag="t1")
                t2 = candpool.tile([128, SC], f32, tag="t2")
                ta = candpool.tile([128, SC], f32, tag="ta")
                nc.vector.tensor_scalar_mul(t1[:], xc, qp[:, 0:1])
                nc.vector.tensor_scalar_mul(t2[:], yc, qp[:, 1:2])
                nc.vector.tensor_add(t1[:], t1[:], t2[:])
                nc.vector.tensor_scalar_mul(t2[:], zc, qp[:, 2:3])
                nc.vector.tensor_add(t1[:], t1[:], t2[:])
                nc.vector.tensor_scalar_mul(t1[:], t1[:], 2.0)
                nc.vector.tensor_scalar_add(ta[:], sqc, qp[:, 3:4])
                # d2n = 2*dot - (sq_q + sq_c) == exact negation of ref d2
                nc.vector.tensor_sub(t1[:], t1[:], ta[:])

                # ---- top-16 of the exact values (2 rounds of max8)
                pos = selpool.tile([128, K_OUT], u32, tag="pos")
                dw = t1
                for r in range(K_OUT // 8):
                    v = selpool.tile([128, 8], f32, tag=f"v{r}")
                    nc.vector.max(out=v[:], in_=dw[:])
                    nc.vector.max_index(
                        out=pos[:, r * 8:(r + 1) * 8], in_max=v[:],
                        in_values=dw[:],
                    )
                    if r < K_OUT // 8 - 1:
                        dn = candpool.tile([128, SC], f32, tag=f"dw{r}")
                        nc.vector.match_replace(
                            out=dn[:], in_to_replace=v[:], in_values=dw[:],
                            imm_value=NEG_BIG,
                        )
                        dw = dn
                pos_i = selpool.tile([128, K_OUT], i32, tag="pos_i")
                nc.vector.tensor_copy(pos_i[:], pos[:])
                nc.sync.dma_start(out=pos_out[qs, :], in_=pos_i[:])

    nc.compile()
    return nc


MLP_DIMS = {
    "base_gate": [2 * C, 64, 32, 1],
    "spatial_attn": [2 * C, 64, 64, 1],
    "geo_encoder": [6, 32, 16],
    "consistency": [2 * C + 16, 64, 32, 1],
    "gate_fusion": [3, 16, 1],
    "confidence": [2 * C, 32, 1],
}


def _w_chunks(din):
    """Row-chunk boundaries for L0 weights; matches the rhs input splits."""
    if din == 3:
        return [(0, 1), (1, 2), (2, 3)]
    if din <= 128:
        return [(0, din)]
    bounds = [0, C, 2 * C] + ([2 * C + 16] if din > 2 * C else [])
    return list(zip(bounds[:-1], bounds[1:]))


def build_phase_b():
    nc = bacc.Bacc("TRN2", target_bir_lowering=False, debug=False,
                   num_devices=N_CORES)
    f2dT = nc.dram_tensor("f2dT", [C, NQ], f32, kind="ExternalInput").ap()
    f3dT = nc.dram_tensor("f3dT", [C, NQ], f32, kind="ExternalInput").ap()
    f23 = nc.dram_tensor("f23", [N, 2 * C], f32, kind="ExternalInput").ap()
    idx16 = nc.dram_tensor("idx16", [NQ, K_OUT], i32,
                           kind="ExternalInput").ap()
    geoT = nc.dram_tensor("geoT", [6, NQ], f32, kind="ExternalInput").ap()
    mv = nc.dram_tensor("mv", [1, NQ], f32, kind="ExternalInput").ap()
    scv = nc.dram_tensor("scv", [1, NQ], f32, kind="ExternalInput").ap()

    wts = {}
    for name, dims in MLP_DIMS.items():
        for li, (din, dout) in enumerate(zip(dims[:-1], dims[1:])):
            for j, (r0, r1) in enumerate(_w_chunks(din)):
                wts[f"{name}_w{li}_k{j}"] = nc.dram_tensor(
                    f"{name}_w{li}_k{j}", [r1 - r0, dout], f32,
                    kind="ExternalInput").ap()
            wts[f"{name}_b{li}"] = nc.dram_tensor(
                f"{name}_b{li}", [dout, 1], f32, kind="ExternalInput").ap()

    fusedT = nc.dram_tensor("fusedT", [C, NQ], f32, kind="ExternalOutput").ap()
    conf_out = nc.dram_tensor("conf_out", [1, NQ], f32,
                              kind="ExternalOutput").ap()

    with TileContext(nc) as tc:
        with (
            tc.tile_pool(name="inp", bufs=1) as inp,
            tc.tile_pool(name="wp", bufs=1) as wp,
            tc.tile_pool(name="act", bufs=2) as actp,
            tc.tile_pool(name="psum", bufs=2, space="PSUM") as psum,
        ):
            f2s = inp.tile([C, NQ], f32, tag="f2s")
            nc.sync.dma_start(out=f2s[:], in_=f2dT[:])
            f3s = inp.tile([C, NQ], f32, tag="f3s")
            nc.sync.dma_start(out=f3s[:], in_=f3dT[:])
            x2s = inp.tile([C, NQ], f32, tag="x2s")
            x3s = inp.tile([C, NQ], f32, tag="x3s")
            ident = inp.tile([128, 128], f32, tag="ident")
            make_identity(nc, ident[:])
            idxs_all = inp.tile([128, K_OUT * QT], i32, tag="idxs_all")
            nc.sync.dma_start(
                out=idxs_all[:].rearrange("p (t k) -> p t k", k=K_OUT),
                in_=idx16[:, :].rearrange("(t p) k -> p t k", p=128))
            # per query-tile: gather 16 neighbor rows of f2d||f3d, mean,
            # transpose, and add to f2dT/f3dT to form the fusion inputs
            with tc.tile_pool(name="gath", bufs=2) as gathp:
                for t in range(QT):
                    tcols = slice(t * 128, (t + 1) * 128)
                    gt = gathp.tile([128, K_OUT * 2 * C], f32, tag="gt")
                    for j in range(K_OUT):
                        nc.gpsimd.indirect_dma_start(
                            out=gt[:, j * 2 * C:(j + 1) * 2 * C],
                            out_offset=None,
                            in_=f23[:, :],
                            in_offset=bass.IndirectOffsetOnAxis(
                                ap=idxs_all[:, t * K_OUT + j:
                                            t * K_OUT + j + 1], axis=0),
                        )
                    mean = gathp.tile([128, 2 * C], f32, tag="mean")
                    nc.vector.tensor_reduce(
                        out=mean[:],
                        in_=gt[:].rearrange("p (k c) -> p c k", k=K_OUT),
                        axis=mybir.AxisListType.X,
                        op=ALU.add,
                    )
                    nc.vector.tensor_scalar_mul(mean[:], mean[:],
                                                1.0 / K_OUT)
                    for half, dst, base in ((0, x2s, f2s), (1, x3s, f3s)):
                        ptT = psum.tile([C, 128], f32, tag="trp")
                        nc.tensor.transpose(
                            ptT[:C, :], mean[:, half * C:(half + 1) * C],
                            ident[:])
                        nc.vector.tensor_add(dst[:, tcols], base[:, tcols],
                                             ptT[:C, :])
            geos = inp.tile([6, NQ], f32, tag="geos")
            nc.sync.dma_start(out=geos[:], in_=geoT[:])
            mvs = inp.tile([1, NQ], f32, tag="mvs")
            nc.sync.dma_start(out=mvs[:], in_=mv[:])
            scs = inp.tile([1, NQ], f32, tag="scs")
            nc.sync.dma_start(out=scs[:], in_=scv[:])
            ones96 = inp.tile([1, C], f32, tag="ones96")
            nc.vector.memset(ones96[:], 1.0)

            ws = {}
            for key, ap_ in wts.items():
                tile_ = wp.tile(list(ap_.shape), f32, tag=key)
                nc.sync.dma_start(out=tile_[:], in_=ap_[:])
                ws[key] = tile_

            def mlp(name, rhs_list, acts):
                """rhs_list: list of already-sliced (k x 512) APs, in the
                same order as the L0 weight row chunks."""
                dims = MLP_DIMS[name]
                h = None
                for li, (din, dout) in enumerate(zip(dims[:-1], dims[1:])):
                    b = ws[f"{name}_b{li}"]
                    pt = psum.tile([dout, 512], f32, tag="mm")
                    if li == 0:
                        n_in = len(rhs_list)
                        assert n_in == len(_w_chunks(din))
                        for j, src in enumerate(rhs_list):
                            nc.tensor.matmul(
                                pt[:dout, :], ws[f"{name}_w0_k{j}"][:], src,
                                start=(j == 0), stop=(j == n_in - 1),
                            )
                    else:
                        nc.tensor.matmul(pt[:dout, :],
                                         ws[f"{name}_w{li}_k0"][:], h[:],
                                         start=True, stop=True)
                    func = acts[li]
                    hn = actp.tile([dout, 512], f32, tag=f"{name}_h{li}")
                    nc.scalar.activation(hn[:], pt[:dout, :], func,
                                         bias=b[:, 0:1])
                    h = hn
                return h

            for ch in range(NQ // 512):
                cs = slice(ch * 512, (ch + 1) * 512)
                bg = mlp("base_gate", [f2s[:, cs], f3s[:, cs]],
                         [AF.Relu, AF.Relu, AF.Sigmoid])
                sg = mlp("spatial_attn", [x2s[:, cs], x3s[:, cs]],
                         [AF.Relu, AF.Relu, AF.Sigmoid])
                cf = mlp("confidence", [f2s[:, cs], f3s[:, cs]],
                         [AF.Relu, AF.Sigmoid])
                ge = mlp("geo_encoder", [geos[:, cs]],
                         [AF.Relu, AF.Relu])
                gg = mlp("consistency",
                         [f2s[:, cs], f3s[:, cs], ge[:]],
                         [AF.Relu, AF.Relu, AF.Sigmoid])

                # gate_fusion L0: w0 is (3 x 16); feed the three 1-row gates
                # as K=1 accumulating matmuls (no partition-offset writes)
                ptf = psum.tile([16, 512], f32, tag="mm")
                nc.tensor.matmul(ptf[:16, :], ws["gate_fusion_w0_k0"][:],
                                 bg[:], start=True, stop=False)
                nc.tensor.matmul(ptf[:16, :], ws["gate_fusion_w0_k1"][:],
                                 sg[:], start=False, stop=False)
                nc.tensor.matmul(ptf[:16, :], ws["gate_fusion_w0_k2"][:],
                                 gg[:], start=False, stop=True)
                fh = actp.tile([16, 512], f32, tag="gf_h0")
                nc.scalar.activation(fh[:], ptf[:16, :], AF.Relu,
                                     bias=ws["gate_fusion_b0"][:, 0:1])
                ptg = psum.tile([1, 512], f32, tag="mm")
                nc.tensor.matmul(ptg[:1, :], ws["gate_fusion_w1_k0"][:],
                                 fh[:], start=True, stop=True)
                fg = actp.tile([1, 512], f32, tag="gf_h1")
                nc.scalar.activation(fg[:], ptg[:1, :], AF.Sigmoid,
                                     bias=ws["gate_fusion_b1"][:, 0:1])

                # valid-mask scaling of the gate and confidence
                fgs = actp.tile([1, 512], f32, tag="fgs")
                nc.vector.tensor_mul(fgs[:], fg[:], mvs[:, cs])
                cfs = actp.tile([1, 512], f32, tag="cfs")
                nc.vector.tensor_mul(cfs[:], cf[:], scs[:, cs])
                nc.sync.dma_start(out=conf_out[:, cs], in_=cfs[:])

                # broadcast gate over 96 channels and mix
                gb = psum.tile([C, 512], f32, tag="gb")
                nc.tensor.matmul(gb[:], ones96[:], fgs[:],
                                 start=True, stop=True)
                om = actp.tile([C, 512], f32, tag="om")
                nc.scalar.activation(om[:], gb[:], AF.Copy,
                                     bias=1.0, scale=-1.0)
                m1 = actp.tile([C, 512], f32, tag="m1")
                nc.vector.tensor_mul(m1[:], gb[:], f2s[:, cs])
                m2 = actp.tile([C, 512], f32, tag="m2")
                nc.vector.tensor_mul(m2[:], om[:], f3s[:, cs])
                nc.vector.tensor_add(m1[:], m1[:], m2[:])
                nc.sync.dma_start(out=fusedT[:, cs], in_=m1[:])

    nc.compile()
    return nc


_CACHE = {}


def _get_programs():
    if "a" not in _CACHE:
        _CACHE["a"] = build_phase_a()
    if "b" not in _CACHE:
        _CACHE["b"] = build_phase_b()
    return _CACHE["a"], _CACHE["b"]


def _mlp_param_maps(params):
    out = {}
    for name, layers in params.items():
        for li, (w, b) in enumerate(layers):
            w = np.asarray(w, np.float32)
            b = np.asarray(b, np.float32)
            wT = np.ascontiguousarray(w.T)
            for j, (r0, r1) in enumerate(_w_chunks(wT.shape[0])):
                out[f"{name}_w{li}_k{j}"] = np.ascontiguousarray(wT[r0:r1])
            out[f"{name}_b{li}"] = np.ascontiguousarray(b[:, None])
    return out


def kernel(f2d, f3d, xyz, valid_mask, params):
    f2d = np.asarray(f2d, np.float32)
    f3d = np.asarray(f3d, np.float32)
    xyz = np.asarray(xyz, np.float32)
    vm = np.asarray(valid_mask)

    nc_a, nc_b = _get_programs()

    x, y, z = xyz[:, 0], xyz[:, 1], xyz[:, 2]
    sq = x * x
    sq = sq + y * y
    sq = sq + z * z
    ones = np.ones(N, np.float32)
    idxf = np.arange(N, dtype=np.float32)
    c5 = np.ascontiguousarray(np.stack([x, y, z, ones, sq]))
    q5_full = np.stack([2.0 * x, 2.0 * y, 2.0 * z, -sq, -ones])
    qpt_full = np.ascontiguousarray(np.stack([x, y, z, sq], axis=1))
    pgt = np.stack([x, y, z, sq, idxf], axis=1).reshape(NG, PG_W)
    pgt = np.ascontiguousarray(pgt)
    f23c = np.ascontiguousarray(np.concatenate([f2d, f3d], axis=1))

    in_maps_a = []
    for cid in range(N_CORES):
        qs = slice(cid * NQ, (cid + 1) * NQ)
        in_maps_a.append({
            "c5": c5,
            "q5": np.ascontiguousarray(q5_full[:, qs]),
            "qpt": np.ascontiguousarray(qpt_full[qs]),
            "pg": pgt,
        })
    import time as _time
    t_a0 = _time.time()
    res_a = run_bass_kernel_spmd(nc_a, in_maps_a, list(range(N_CORES)))
    t_a = _time.time() - t_a0

    pos = np.concatenate([res_a.results[c]["pos_out"]
                          for c in range(N_CORES)], axis=0)
    gidx_all = np.concatenate([res_a.results[c]["gidx_out"]
                               for c in range(N_CORES)], axis=0)
    # absolute neighbor index = selected_group * G + offset_within_group
    grp = np.take_along_axis(gidx_all, pos >> 4, axis=1)
    idx = (grp * G + (pos & 15)).astype(np.int32)
    globals()["_last_idx"] = idx
    globals()["_t_phase_a"] = t_a

    # ---- host: normal estimation via eigh on CPU (matches reference impl)
    import jax
    import jax.numpy as jnp
    cpu = jax.devices("cpu")[0]
    idx_nm = idx[:, 1:9]
    with jax.default_device(cpu):
        jxyz = jnp.asarray(xyz)
        nbr = jxyz[jnp.asarray(idx_nm)]
        centered = nbr - jxyz[:, None, :]
        cov = jnp.einsum('nkd,nke->nde', centered, centered)
        _, evecs = jnp.linalg.eigh(cov)
        normals = evecs[:, :, 0]
        normals = normals / jnp.maximum(
            jnp.linalg.norm(normals, axis=1, keepdims=True), 1e-12)
        normals = np.asarray(normals)

    vw = vm.astype(np.float32)
    mvv = vw + np.float32(0.1) * (np.float32(1.0) - vw)
    scvv = vw * np.float32(0.9) + np.float32(0.1)
    geo = np.concatenate([xyz, normals], axis=1)

    wmaps = _mlp_param_maps(params)
    in_maps_b = []
    for cid in range(N_CORES):
        qs = slice(cid * NQ, (cid + 1) * NQ)
        m = {
            "f2dT": np.ascontiguousarray(f2d[qs].T),
            "f3dT": np.ascontiguousarray(f3d[qs].T),
            "f23": f23c,
            "idx16": np.ascontiguousarray(idx[qs]),
            "geoT": np.ascontiguousarray(geo[qs].T),
            "mv": np.ascontiguousarray(mvv[qs][None, :]),
            "scv": np.ascontiguousarray(scvv[qs][None, :]),
        }
        m.update(wmaps)
        in_maps_b.append(m)
    t_b0 = _time.time()
    res_b = run_bass_kernel_spmd(nc_b, in_maps_b, list(range(N_CORES)))
    globals()["_t_phase_b"] = _time.time() - t_b0

    fused = np.concatenate([res_b.results[c]["fusedT"].T
                            for c in range(N_CORES)], axis=0)
    conf = np.concatenate([res_b.results[c]["conf_out"].T
                           for c in range(N_CORES)], axis=0)
    return np.ascontiguousarray(fused), np.ascontiguousarray(conf)


# revision 45
# speedup vs baseline: 1.0128x; 1.0128x over previous
"""Trainium2 Bass kernel for nn_EnhancedFusionGate (kNN fusion gate).

Data-parallel over points: each of the 8 cores handles 2048 of the 16384
rows. Phase A finds exact 16-NN per point via a grouped-max pruning pass
(PE fp32r distance matmul + DVE max8 machinery) and computes neighbor
feature means. The 3x3 eigendecompositions run on host CPU (bit-matching
the reference LAPACK path). Phase B runs all MLP gates and the final mix.
"""

import numpy as np

import concourse.bacc as bacc
import concourse.bass as bass
import concourse.mybir as mybir
from concourse.tile import TileContext
from concourse.bass_utils import run_bass_kernel_spmd
from concourse.masks import make_identity

N = 16384
C = 96
N_CORES = 8
NQ = N // N_CORES            # 2048 queries per core
QT = NQ // 128               # 16 query tiles per core
G = 16                       # candidates per pruning group
NG = N // G                  # 1024 groups
SEL_G = 24                   # groups gathered per query (3 max8 rounds)
SC = SEL_G * G               # 384 exact candidates per query
CAND_W = 4                   # x, y, z, sq per candidate
PG_W = G * CAND_W            # 80 floats per group row
K_OUT = 16                   # neighbors returned
NJB = 8                      # phase-B dma_gather blocks per core
QPB = NQ // NJB              # queries per gather block (256)
NEG_BIG = float(np.float32(-3.0e38))

f32 = mybir.dt.float32
f32r = mybir.dt.float32r
i32 = mybir.dt.int32
u32 = mybir.dt.uint32

AF = mybir.ActivationFunctionType
ALU = mybir.AluOpType


def build_phase_a():
    nc = bacc.Bacc("TRN2", target_bir_lowering=False, debug=False,
                   num_devices=N_CORES)
    c5 = nc.dram_tensor("c5", [5, N], f32, kind="ExternalInput").ap()
    q5 = nc.dram_tensor("q5", [5, NQ], f32, kind="ExternalInput").ap()
    qpt = nc.dram_tensor("qpt", [NQ, 4], f32, kind="ExternalInput").ap()
    pg = nc.dram_tensor("pg", [NG, PG_W], f32, kind="ExternalInput").ap()

    pos_out = nc.dram_tensor("pos_out", [NQ, K_OUT], i32,
                             kind="ExternalOutput").ap()
    gidx_out = nc.dram_tensor("gidx_out", [NQ, SEL_G], i32,
                              kind="ExternalOutput").ap()

    bf16 = mybir.dt.bfloat16
    # chunks whose seg-max runs via an ACT bf16 cast-copy instead of a
    # direct DVE read from PSUM (balances the two engines)
    N_CH = N // 512
    act_chunks = set(range(0, N_CH, 2))

    with TileContext(nc) as tc:
        with (
            tc.tile_pool(name="cpool", bufs=1) as cpool,
            tc.tile_pool(name="qpool", bufs=2) as qpool,
            tc.tile_pool(name="gmpool", bufs=2) as gmpool,
            tc.tile_pool(name="selpool", bufs=2) as selpool,
            tc.tile_pool(name="candpool", bufs=2) as candpool,
            tc.tile_pool(name="bfpool", bufs=3) as bfpool,
            tc.tile_pool(name="psum", bufs=8, space="PSUM") as psum,
        ):
            c5s = cpool.tile([5, N], f32r, tag="c5s")
            nc.sync.dma_start(out=c5s[:], in_=c5[:].bitcast(f32r))
            q5s = cpool.tile([5, NQ], f32r, tag="q5s")
            nc.sync.dma_start(out=q5s[:], in_=q5[:].bitcast(f32r))

            for t in range(QT):
                qs = slice(t * 128, (t + 1) * 128)
                # per-query point data (x, y, z, sq) for exact recompute
                qp = qpool.tile([128, 4], f32, tag="qp")
                nc.sync.dma_start(out=qp[:], in_=qpt[qs, :])

                # ---- pruning pass: group maxes of -d2 over all candidates
                gm = gmpool.tile([128, NG], f32, tag="gm")
                for ch in range(N_CH):
                    pt = psum.tile([128, 512], f32, tag="negd2")
                    nc.tensor.matmul(
                        pt[:],
                        q5s[:, qs],
                        c5s[:, ch * 512:(ch + 1) * 512],
                        start=True, stop=True,
                    )
                    if ch in act_chunks:
                        bt = bfpool.tile([128, 512], bf16, tag="bt")
                        nc.scalar.copy(bt[:], pt[:])
                        red_in = bt[:].rearrange("p (g k) -> p g k", k=G)
                    else:
                        red_in = pt[:].rearrange("p (g k) -> p g k", k=G)
                    nc.vector.tensor_reduce(
                        out=gm[:, ch * 32:(ch + 1) * 32],
                        in_=red_in,
                        axis=mybir.AxisListType.X,
                        op=ALU.max,
                    )

                # ---- pick top SEL_G groups by group max (3 rounds of max8)
                gidx = selpool.tile([128, SEL_G], u32, tag="gidx")
                gmw = gm
                for r in range(SEL_G // 8):
                    gv = selpool.tile([128, 8], f32, tag=f"gv{r}")
                    nc.vector.max(out=gv[:], in_=gmw[:])
                    nc.vector.max_index(
                        out=gidx[:, r * 8:(r + 1) * 8], in_max=gv[:],
                        in_values=gmw[:],
                    )
                    if r < SEL_G // 8 - 1:
                        gmn = gmpool.tile([128, NG], f32, tag=f"gmr{r}")
                        nc.vector.match_replace(
                            out=gmn[:], in_to_replace=gv[:], in_values=gmw[:],
                            imm_value=NEG_BIG,
                        )
                        gmw = gmn
                gidx_i = selpool.tile([128, SEL_G], i32, tag="gidx_i")
                nc.vector.tensor_copy(gidx_i[:], gidx[:])
                nc.sync.dma_start(out=gidx_out[qs, :], in_=gidx_i[:])

                # ---- gather the SEL_G groups' candidate data from DRAM
                # (hardware indirect DMA honors one offset per partition,
                # so one DMA per group slot)
                cand = candpool.tile([128, SEL_G * PG_W], f32, tag="cand")
                for s in range(SEL_G):
                    nc.gpsimd.indirect_dma_start(
                        out=cand[:, s * PG_W:(s + 1) * PG_W],
                        out_offset=None,
                        in_=pg[:, :],
                        in_offset=bass.IndirectOffsetOnAxis(
                            ap=gidx[:, s:s + 1], axis=0),
                    )
                cpl = cand[:].rearrange("p (s v) -> p s v", v=CAND_W)
                xc, yc = cpl[:, :, 0], cpl[:, :, 1]
                zc, sqc = cpl[:, :, 2], cpl[:, :, 3]

                # ---- exact -d2 recompute, mirroring the reference fp32
                # ops (fused: (in0 op0 scalar) op1 in1; all reorderings
                # used here are bitwise-neutral commutations)
                t1 = candpool.tile([128, SC], f32, tag="t1")
                ta = candpool.tile([128, SC], f32, tag="ta")
                nc.vector.tensor_scalar_mul(t1[:], xc, qp[:, 0:1])
                nc.vector.scalar_tensor_tensor(
                    t1[:], yc, qp[:, 1:2], t1[:],
                    op0=ALU.mult, op1=ALU.add)
                nc.vector.scalar_tensor_tensor(
                    t1[:], zc, qp[:, 2:3], t1[:],
                    op0=ALU.mult, op1=ALU.add)
                nc.vector.tensor_scalar_add(ta[:], sqc, qp[:, 3:4])
                # d2n = 2*dot - (sq_q + sq_c) == exact negation of ref d2
                nc.vector.scalar_tensor_tensor(
                    t1[:], t1[:], 2.0, ta[:],
                    op0=ALU.mult, op1=ALU.subtract)

                # ---- top-16 of the exact values (2 rounds of max8)
                pos = selpool.tile([128, K_OUT], u32, tag="pos")
                dw = t1
                for r in range(K_OUT // 8):
                    v = selpool.tile([128, 8], f32, tag=f"v{r}")
                    nc.vector.max(out=v[:], in_=dw[:])
                    nc.vector.max_index(
                        out=pos[:, r * 8:(r + 1) * 8], in_max=v[:],
                        in_values=dw[:],
                    )
                    if r < K_OUT // 8 - 1:
                        dn = candpool.tile([128, SC], f32, tag=f"dw{r}")
                        nc.vector.match_replace(
                            out=dn[:], in_to_replace=v[:], in_values=dw[:],
                            imm_value=NEG_BIG,
                        )
                        dw = dn
                pos_i = selpool.tile([128, K_OUT], i32, tag="pos_i")
                nc.vector.tensor_copy(pos_i[:], pos[:])
                nc.sync.dma_start(out=pos_out[qs, :], in_=pos_i[:])

    nc.compile()
    return nc


MLP_DIMS = {
    "base_gate": [2 * C, 64, 32, 1],
    "spatial_attn": [2 * C, 64, 64, 1],
    "geo_encoder": [6, 32, 16],
    "consistency": [2 * C + 16, 64, 32, 1],
    "gate_fusion": [3, 16, 1],
    "confidence": [2 * C, 32, 1],
}


def _w_chunks(din):
    """Row-chunk boundaries for L0 weights; matches the rhs input splits."""
    if din == 3:
        return [(0, 1), (1, 2), (2, 3)]
    if din <= 128:
        return [(0, din)]
    bounds = [0, C, 2 * C] + ([2 * C + 16] if din > 2 * C else [])
    return list(zip(bounds[:-1], bounds[1:]))


def build_phase_b():
    nc = bacc.Bacc("TRN2", target_bir_lowering=False, debug=False,
                   num_devices=N_CORES)
    f2dT = nc.dram_tensor("f2dT", [C, NQ], f32, kind="ExternalInput").ap()
    f3dT = nc.dram_tensor("f3dT", [C, NQ], f32, kind="ExternalInput").ap()
    f23 = nc.dram_tensor("f23", [N, 2 * C], f32, kind="ExternalInput").ap()
    idx16 = nc.dram_tensor("idx16", [NQ, K_OUT], i32,
                           kind="ExternalInput").ap()
    geoT = nc.dram_tensor("geoT", [6, NQ], f32, kind="ExternalInput").ap()
    mv = nc.dram_tensor("mv", [1, NQ], f32, kind="ExternalInput").ap()
    scv = nc.dram_tensor("scv", [1, NQ], f32, kind="ExternalInput").ap()

    wts = {}
    for name, dims in MLP_DIMS.items():
        for li, (din, dout) in enumerate(zip(dims[:-1], dims[1:])):
            for j, (r0, r1) in enumerate(_w_chunks(din)):
                wts[f"{name}_w{li}_k{j}"] = nc.dram_tensor(
                    f"{name}_w{li}_k{j}", [r1 - r0, dout], f32,
                    kind="ExternalInput").ap()
            wts[f"{name}_b{li}"] = nc.dram_tensor(
                f"{name}_b{li}", [dout, 1], f32, kind="ExternalInput").ap()

    fusedT = nc.dram_tensor("fusedT", [C, NQ], f32, kind="ExternalOutput").ap()
    conf_out = nc.dram_tensor("conf_out", [1, NQ], f32,
                              kind="ExternalOutput").ap()

    with TileContext(nc) as tc:
        with (
            tc.tile_pool(name="inp", bufs=1) as inp,
            tc.tile_pool(name="wp", bufs=1) as wp,
            tc.tile_pool(name="act", bufs=2) as actp,
            tc.tile_pool(name="psum", bufs=2, space="PSUM") as psum,
        ):
            f2s = inp.tile([C, NQ], f32, tag="f2s")
            nc.sync.dma_start(out=f2s[:], in_=f2dT[:])
            f3s = inp.tile([C, NQ], f32, tag="f3s")
            nc.sync.dma_start(out=f3s[:], in_=f3dT[:])
            x2s = inp.tile([C, NQ], f32, tag="x2s")
            x3s = inp.tile([C, NQ], f32, tag="x3s")
            ident = inp.tile([128, 128], f32, tag="ident")
            make_identity(nc, ident[:])
            idxs_all = inp.tile([128, K_OUT * QT], i32, tag="idxs_all")
            nc.sync.dma_start(
                out=idxs_all[:].rearrange("p (t k) -> p t k", k=K_OUT),
                in_=idx16[:, :].rearrange("(t p) k -> p t k", p=128))
            # per query-tile: gather 16 neighbor rows of f2d||f3d, mean,
            # transpose, and add to f2dT/f3dT to form the fusion inputs
            with tc.tile_pool(name="gath", bufs=2) as gathp:
                for t in range(QT):
                    tcols = slice(t * 128, (t + 1) * 128)
                    gt = gathp.tile([128, K_OUT * 2 * C], f32, tag="gt")
                    for j in range(K_OUT):
                        nc.gpsimd.indirect_dma_start(
                            out=gt[:, j * 2 * C:(j + 1) * 2 * C],
                            out_offset=None,
                            in_=f23[:, :],
                            in_offset=bass.IndirectOffsetOnAxis(
                                ap=idxs_all[:, t * K_OUT + j:
                                            t * K_OUT + j + 1], axis=0),
                        )
                    mean = gathp.tile([128, 2 * C], f32, tag="mean")
                    nc.vector.tensor_reduce(
                        out=mean[:],
                        in_=gt[:].rearrange("p (k c) -> p c k", k=K_OUT),
                        axis=mybir.AxisListType.X,
                        op=ALU.add,
                    )
                    nc.vector.tensor_scalar_mul(mean[:], mean[:],
                                                1.0 / K_OUT)
                    for half, dst, base in ((0, x2s, f2s), (1, x3s, f3s)):
                        ptT = psum.tile([C, 128], f32, tag="trp")
                        nc.tensor.transpose(
                            ptT[:C, :], mean[:, half * C:(half + 1) * C],
                            ident[:])
                        nc.vector.tensor_add(dst[:, tcols], base[:, tcols],
                                             ptT[:C, :])
            geos = inp.tile([6, NQ], f32, tag="geos")
            nc.sync.dma_start(out=geos[:], in_=geoT[:])
            mvs = inp.tile([1, NQ], f32, tag="mvs")
            nc.sync.dma_start(out=mvs[:], in_=mv[:])
            scs = inp.tile([1, NQ], f32, tag="scs")
            nc.sync.dma_start(out=scs[:], in_=scv[:])
            ones96 = inp.tile([1, C], f32, tag="ones96")
            nc.vector.memset(ones96[:], 1.0)

            ws = {}
            for key, ap_ in wts.items():
                tile_ = wp.tile(list(ap_.shape), f32, tag=key)
                nc.sync.dma_start(out=tile_[:], in_=ap_[:])
                ws[key] = tile_

            def mlp(name, rhs_list, acts):
                """rhs_list: list of already-sliced (k x 512) APs, in the
                same order as the L0 weight row chunks."""
                dims = MLP_DIMS[name]
                h = None
                for li, (din, dout) in enumerate(zip(dims[:-1], dims[1:])):
                    b = ws[f"{name}_b{li}"]
                    pt = psum.tile([dout, 512], f32, tag="mm")
                    if li == 0:
                        n_in = len(rhs_list)
                        assert n_in == len(_w_chunks(din))
                        for j, src in enumerate(rhs_list):
                            nc.tensor.matmul(
                                pt[:dout, :], ws[f"{name}_w0_k{j}"][:], src,
                                start=(j == 0), stop=(j == n_in - 1),
                            )
                    else:
                        nc.tensor.matmul(pt[:dout, :],
                                         ws[f"{name}_w{li}_k0"][:], h[:],
                                         start=True, stop=True)
                    func = acts[li]
                    hn = actp.tile([dout, 512], f32, tag=f"{name}_h{li}")
                    nc.scalar.activation(hn[:], pt[:dout, :], func,
                                         bias=b[:, 0:1])
                    h = hn
                return h

            for ch in range(NQ // 512):
                cs = slice(ch * 512, (ch + 1) * 512)
                bg = mlp("base_gate", [f2s[:, cs], f3s[:, cs]],
                         [AF.Relu, AF.Relu, AF.Sigmoid])
                sg = mlp("spatial_attn", [x2s[:, cs], x3s[:, cs]],
                         [AF.Relu, AF.Relu, AF.Sigmoid])
                cf = mlp("confidence", [f2s[:, cs], f3s[:, cs]],
                         [AF.Relu, AF.Sigmoid])
                ge = mlp("geo_encoder", [geos[:, cs]],
                         [AF.Relu, AF.Relu])
                gg = mlp("consistency",
                         [f2s[:, cs], f3s[:, cs], ge[:]],
                         [AF.Relu, AF.Relu, AF.Sigmoid])

                # gate_fusion L0: w0 is (3 x 16); feed the three 1-row gates
                # as K=1 accumulating matmuls (no partition-offset writes)
                ptf = psum.tile([16, 512], f32, tag="mm")
                nc.tensor.matmul(ptf[:16, :], ws["gate_fusion_w0_k0"][:],
                                 bg[:], start=True, stop=False)
                nc.tensor.matmul(ptf[:16, :], ws["gate_fusion_w0_k1"][:],
                                 sg[:], start=False, stop=False)
                nc.tensor.matmul(ptf[:16, :], ws["gate_fusion_w0_k2"][:],
                                 gg[:], start=False, stop=True)
                fh = actp.tile([16, 512], f32, tag="gf_h0")
                nc.scalar.activation(fh[:], ptf[:16, :], AF.Relu,
                                     bias=ws["gate_fusion_b0"][:, 0:1])
                ptg = psum.tile([1, 512], f32, tag="mm")
                nc.tensor.matmul(ptg[:1, :], ws["gate_fusion_w1_k0"][:],
                                 fh[:], start=True, stop=True)
                fg = actp.tile([1, 512], f32, tag="gf_h1")
                nc.scalar.activation(fg[:], ptg[:1, :], AF.Sigmoid,
                                     bias=ws["gate_fusion_b1"][:, 0:1])

                # valid-mask scaling of the gate and confidence
                fgs = actp.tile([1, 512], f32, tag="fgs")
                nc.vector.tensor_mul(fgs[:], fg[:], mvs[:, cs])
                cfs = actp.tile([1, 512], f32, tag="cfs")
                nc.vector.tensor_mul(cfs[:], cf[:], scs[:, cs])
                nc.sync.dma_start(out=conf_out[:, cs], in_=cfs[:])

                # broadcast gate over 96 channels and mix
                gb = psum.tile([C, 512], f32, tag="gb")
                nc.tensor.matmul(gb[:], ones96[:], fgs[:],
                                 start=True, stop=True)
                om = actp.tile([C, 512], f32, tag="om")
                nc.scalar.activation(om[:], gb[:], AF.Copy,
                                     bias=1.0, scale=-1.0)
                m1 = actp.tile([C, 512], f32, tag="m1")
                nc.vector.tensor_mul(m1[:], gb[:], f2s[:, cs])
                m2 = actp.tile([C, 512], f32, tag="m2")
                nc.vector.tensor_mul(m2[:], om[:], f3s[:, cs])
                nc.vector.tensor_add(m1[:], m1[:], m2[:])
                nc.sync.dma_start(out=fusedT[:, cs], in_=m1[:])

    nc.compile()
    return nc


_CACHE = {}


def _get_programs():
    if "a" not in _CACHE:
        _CACHE["a"] = build_phase_a()
    if "b" not in _CACHE:
        _CACHE["b"] = build_phase_b()
    return _CACHE["a"], _CACHE["b"]


def _mlp_param_maps(params):
    out = {}
    for name, layers in params.items():
        for li, (w, b) in enumerate(layers):
            w = np.asarray(w, np.float32)
            b = np.asarray(b, np.float32)
            wT = np.ascontiguousarray(w.T)
            for j, (r0, r1) in enumerate(_w_chunks(wT.shape[0])):
                out[f"{name}_w{li}_k{j}"] = np.ascontiguousarray(wT[r0:r1])
            out[f"{name}_b{li}"] = np.ascontiguousarray(b[:, None])
    return out


def kernel(f2d, f3d, xyz, valid_mask, params):
    f2d = np.asarray(f2d, np.float32)
    f3d = np.asarray(f3d, np.float32)
    xyz = np.asarray(xyz, np.float32)
    vm = np.asarray(valid_mask)

    nc_a, nc_b = _get_programs()

    x, y, z = xyz[:, 0], xyz[:, 1], xyz[:, 2]
    sq = x * x
    sq = sq + y * y
    sq = sq + z * z
    ones = np.ones(N, np.float32)
    c5 = np.ascontiguousarray(np.stack([x, y, z, ones, sq]))
    q5_full = np.stack([2.0 * x, 2.0 * y, 2.0 * z, -sq, -ones])
    qpt_full = np.ascontiguousarray(np.stack([x, y, z, sq], axis=1))
    pgt = np.stack([x, y, z, sq], axis=1).reshape(NG, PG_W)
    pgt = np.ascontiguousarray(pgt)
    f23c = np.ascontiguousarray(np.concatenate([f2d, f3d], axis=1))

    in_maps_a = []
    for cid in range(N_CORES):
        qs = slice(cid * NQ, (cid + 1) * NQ)
        in_maps_a.append({
            "c5": c5,
            "q5": np.ascontiguousarray(q5_full[:, qs]),
            "qpt": np.ascontiguousarray(qpt_full[qs]),
            "pg": pgt,
        })
    import time as _time
    t_a0 = _time.time()
    res_a = run_bass_kernel_spmd(nc_a, in_maps_a, list(range(N_CORES)))
    t_a = _time.time() - t_a0

    pos = np.concatenate([res_a.results[c]["pos_out"]
                          for c in range(N_CORES)], axis=0)
    gidx_all = np.concatenate([res_a.results[c]["gidx_out"]
                               for c in range(N_CORES)], axis=0)
    # absolute neighbor index = selected_group * G + offset_within_group
    grp = np.take_along_axis(gidx_all, pos >> 4, axis=1)
    idx = (grp * G + (pos & 15)).astype(np.int32)
    globals()["_last_idx"] = idx
    globals()["_t_phase_a"] = t_a

    # ---- host: normal estimation via eigh on CPU (matches reference impl)
    import jax
    import jax.numpy as jnp
    cpu = jax.devices("cpu")[0]
    idx_nm = idx[:, 1:9]
    with jax.default_device(cpu):
        jxyz = jnp.asarray(xyz)
        nbr = jxyz[jnp.asarray(idx_nm)]
        centered = nbr - jxyz[:, None, :]
        cov = jnp.einsum('nkd,nke->nde', centered, centered)
        _, evecs = jnp.linalg.eigh(cov)
        normals = evecs[:, :, 0]
        normals = normals / jnp.maximum(
            jnp.linalg.norm(normals, axis=1, keepdims=True), 1e-12)
        normals = np.asarray(normals)

    vw = vm.astype(np.float32)
    mvv = vw + np.float32(0.1) * (np.float32(1.0) - vw)
    scvv = vw * np.float32(0.9) + np.float32(0.1)
    geo = np.concatenate([xyz, normals], axis=1)

    wmaps = _mlp_param_maps(params)
    in_maps_b = []
    for cid in range(N_CORES):
        qs = slice(cid * NQ, (cid + 1) * NQ)
        m = {
            "f2dT": np.ascontiguousarray(f2d[qs].T),
            "f3dT": np.ascontiguousarray(f3d[qs].T),
            "f23": f23c,
            "idx16": np.ascontiguousarray(idx[qs]),
            "geoT": np.ascontiguousarray(geo[qs].T),
            "mv": np.ascontiguousarray(mvv[qs][None, :]),
            "scv": np.ascontiguousarray(scvv[qs][None, :]),
        }
        m.update(wmaps)
        in_maps_b.append(m)
    t_b0 = _time.time()
    res_b = run_bass_kernel_spmd(nc_b, in_maps_b, list(range(N_CORES)))
    globals()["_t_phase_b"] = _time.time() - t_b0

    fused = np.concatenate([res_b.results[c]["fusedT"].T
                            for c in range(N_CORES)], axis=0)
    conf = np.concatenate([res_b.results[c]["conf_out"].T
                           for c in range(N_CORES)], axis=0)
    return np.ascontiguousarray(fused), np.ascontiguousarray(conf)


# revision 46
# speedup vs baseline: 1.0207x; 1.0078x over previous
"""Trainium2 Bass kernel for nn_EnhancedFusionGate (kNN fusion gate).

Data-parallel over points: each of the 8 cores handles 2048 of the 16384
rows. Phase A finds exact 16-NN per point via a grouped-max pruning pass
(PE fp32r distance matmul + DVE max8 machinery) and computes neighbor
feature means. The 3x3 eigendecompositions run on host CPU (bit-matching
the reference LAPACK path). Phase B runs all MLP gates and the final mix.
"""

import numpy as np

import concourse.bacc as bacc
import concourse.bass as bass
import concourse.mybir as mybir
from concourse.tile import TileContext
from concourse.bass_utils import run_bass_kernel_spmd
from concourse.masks import make_identity

N = 16384
C = 96
N_CORES = 8
NQ = N // N_CORES            # 2048 queries per core
QT = NQ // 128               # 16 query tiles per core
G = 16                       # candidates per pruning group
NG = N // G                  # 1024 groups
SEL_G = 24                   # groups gathered per query (3 max8 rounds)
SC = SEL_G * G               # 384 exact candidates per query
CAND_W = 4                   # x, y, z, sq per candidate
PG_W = G * CAND_W            # 80 floats per group row
K_OUT = 16                   # neighbors returned
NJB = 8                      # phase-B dma_gather blocks per core
QPB = NQ // NJB              # queries per gather block (256)
NEG_BIG = float(np.float32(-3.0e38))

f32 = mybir.dt.float32
f32r = mybir.dt.float32r
i32 = mybir.dt.int32
u32 = mybir.dt.uint32

AF = mybir.ActivationFunctionType
ALU = mybir.AluOpType


def build_phase_a():
    nc = bacc.Bacc("TRN2", target_bir_lowering=False, debug=False,
                   num_devices=N_CORES)
    c5 = nc.dram_tensor("c5", [5, N], f32, kind="ExternalInput").ap()
    q5 = nc.dram_tensor("q5", [5, NQ], f32, kind="ExternalInput").ap()
    qpt = nc.dram_tensor("qpt", [NQ, 4], f32, kind="ExternalInput").ap()
    pg = nc.dram_tensor("pg", [NG, PG_W], f32, kind="ExternalInput").ap()

    pos_out = nc.dram_tensor("pos_out", [NQ, K_OUT], i32,
                             kind="ExternalOutput").ap()
    gidx_out = nc.dram_tensor("gidx_out", [NQ, SEL_G], i32,
                              kind="ExternalOutput").ap()

    bf16 = mybir.dt.bfloat16
    # chunks whose seg-max runs via an ACT bf16 cast-copy instead of a
    # direct DVE read from PSUM (balances the two engines)
    N_CH = N // 512
    act_chunks = set(range(N_CH))

    with TileContext(nc) as tc:
        with (
            tc.tile_pool(name="cpool", bufs=1) as cpool,
            tc.tile_pool(name="qpool", bufs=2) as qpool,
            tc.tile_pool(name="gmpool", bufs=2) as gmpool,
            tc.tile_pool(name="selpool", bufs=2) as selpool,
            tc.tile_pool(name="candpool", bufs=2) as candpool,
            tc.tile_pool(name="bfpool", bufs=3) as bfpool,
            tc.tile_pool(name="psum", bufs=8, space="PSUM") as psum,
        ):
            c5s = cpool.tile([5, N], f32r, tag="c5s")
            nc.sync.dma_start(out=c5s[:], in_=c5[:].bitcast(f32r))
            q5s = cpool.tile([5, NQ], f32r, tag="q5s")
            nc.sync.dma_start(out=q5s[:], in_=q5[:].bitcast(f32r))

            for t in range(QT):
                qs = slice(t * 128, (t + 1) * 128)
                # per-query point data (x, y, z, sq) for exact recompute
                qp = qpool.tile([128, 4], f32, tag="qp")
                nc.sync.dma_start(out=qp[:], in_=qpt[qs, :])

                # ---- pruning pass: group maxes of -d2 over all candidates
                gm = gmpool.tile([128, NG], f32, tag="gm")
                for ch in range(N_CH):
                    pt = psum.tile([128, 512], f32, tag="negd2")
                    nc.tensor.matmul(
                        pt[:],
                        q5s[:, qs],
                        c5s[:, ch * 512:(ch + 1) * 512],
                        start=True, stop=True,
                    )
                    if ch in act_chunks:
                        bt = bfpool.tile([128, 512], bf16, tag="bt")
                        nc.scalar.copy(bt[:], pt[:])
                        red_in = bt[:].rearrange("p (g k) -> p g k", k=G)
                    else:
                        red_in = pt[:].rearrange("p (g k) -> p g k", k=G)
                    nc.vector.tensor_reduce(
                        out=gm[:, ch * 32:(ch + 1) * 32],
                        in_=red_in,
                        axis=mybir.AxisListType.X,
                        op=ALU.max,
                    )

                # ---- pick top SEL_G groups by group max (3 rounds of max8)
                gidx = selpool.tile([128, SEL_G], u32, tag="gidx")
                gmw = gm
                for r in range(SEL_G // 8):
                    gv = selpool.tile([128, 8], f32, tag=f"gv{r}")
                    nc.vector.max(out=gv[:], in_=gmw[:])
                    nc.vector.max_index(
                        out=gidx[:, r * 8:(r + 1) * 8], in_max=gv[:],
                        in_values=gmw[:],
                    )
                    if r < SEL_G // 8 - 1:
                        gmn = gmpool.tile([128, NG], f32, tag=f"gmr{r}")
                        nc.vector.match_replace(
                            out=gmn[:], in_to_replace=gv[:], in_values=gmw[:],
                            imm_value=NEG_BIG,
                        )
                        gmw = gmn
                gidx_i = selpool.tile([128, SEL_G], i32, tag="gidx_i")
                nc.vector.tensor_copy(gidx_i[:], gidx[:])
                nc.sync.dma_start(out=gidx_out[qs, :], in_=gidx_i[:])

                # ---- gather the SEL_G groups' candidate data from DRAM
                # (hardware indirect DMA honors one offset per partition,
                # so one DMA per group slot)
                cand = candpool.tile([128, SEL_G * PG_W], f32, tag="cand")
                for s in range(SEL_G):
                    nc.gpsimd.indirect_dma_start(
                        out=cand[:, s * PG_W:(s + 1) * PG_W],
                        out_offset=None,
                        in_=pg[:, :],
                        in_offset=bass.IndirectOffsetOnAxis(
                            ap=gidx[:, s:s + 1], axis=0),
                    )
                cpl = cand[:].rearrange("p (s v) -> p s v", v=CAND_W)
                xc, yc = cpl[:, :, 0], cpl[:, :, 1]
                zc, sqc = cpl[:, :, 2], cpl[:, :, 3]

                # ---- exact -d2 recompute, mirroring the reference fp32
                # ops (fused: (in0 op0 scalar) op1 in1; all reorderings
                # used here are bitwise-neutral commutations)
                t1 = candpool.tile([128, SC], f32, tag="t1")
                ta = candpool.tile([128, SC], f32, tag="ta")
                nc.vector.tensor_scalar_mul(t1[:], xc, qp[:, 0:1])
                nc.vector.scalar_tensor_tensor(
                    t1[:], yc, qp[:, 1:2], t1[:],
                    op0=ALU.mult, op1=ALU.add)
                nc.vector.scalar_tensor_tensor(
                    t1[:], zc, qp[:, 2:3], t1[:],
                    op0=ALU.mult, op1=ALU.add)
                nc.vector.tensor_scalar_add(ta[:], sqc, qp[:, 3:4])
                # d2n = 2*dot - (sq_q + sq_c) == exact negation of ref d2
                nc.vector.scalar_tensor_tensor(
                    t1[:], t1[:], 2.0, ta[:],
                    op0=ALU.mult, op1=ALU.subtract)

                # ---- top-16 of the exact values (2 rounds of max8)
                pos = selpool.tile([128, K_OUT], u32, tag="pos")
                dw = t1
                for r in range(K_OUT // 8):
                    v = selpool.tile([128, 8], f32, tag=f"v{r}")
                    nc.vector.max(out=v[:], in_=dw[:])
                    nc.vector.max_index(
                        out=pos[:, r * 8:(r + 1) * 8], in_max=v[:],
                        in_values=dw[:],
                    )
                    if r < K_OUT // 8 - 1:
                        dn = candpool.tile([128, SC], f32, tag=f"dw{r}")
                        nc.vector.match_replace(
                            out=dn[:], in_to_replace=v[:], in_values=dw[:],
                            imm_value=NEG_BIG,
                        )
                        dw = dn
                pos_i = selpool.tile([128, K_OUT], i32, tag="pos_i")
                nc.vector.tensor_copy(pos_i[:], pos[:])
                nc.sync.dma_start(out=pos_out[qs, :], in_=pos_i[:])

    nc.compile()
    return nc


MLP_DIMS = {
    "base_gate": [2 * C, 64, 32, 1],
    "spatial_attn": [2 * C, 64, 64, 1],
    "geo_encoder": [6, 32, 16],
    "consistency": [2 * C + 16, 64, 32, 1],
    "gate_fusion": [3, 16, 1],
    "confidence": [2 * C, 32, 1],
}


def _w_chunks(din):
    """Row-chunk boundaries for L0 weights; matches the rhs input splits."""
    if din == 3:
        return [(0, 1), (1, 2), (2, 3)]
    if din <= 128:
        return [(0, din)]
    bounds = [0, C, 2 * C] + ([2 * C + 16] if din > 2 * C else [])
    return list(zip(bounds[:-1], bounds[1:]))


def build_phase_b():
    nc = bacc.Bacc("TRN2", target_bir_lowering=False, debug=False,
                   num_devices=N_CORES)
    f2dT = nc.dram_tensor("f2dT", [C, NQ], f32, kind="ExternalInput").ap()
    f3dT = nc.dram_tensor("f3dT", [C, NQ], f32, kind="ExternalInput").ap()
    f23 = nc.dram_tensor("f23", [N, 2 * C], f32, kind="ExternalInput").ap()
    idx16 = nc.dram_tensor("idx16", [NQ, K_OUT], i32,
                           kind="ExternalInput").ap()
    geoT = nc.dram_tensor("geoT", [6, NQ], f32, kind="ExternalInput").ap()
    mv = nc.dram_tensor("mv", [1, NQ], f32, kind="ExternalInput").ap()
    scv = nc.dram_tensor("scv", [1, NQ], f32, kind="ExternalInput").ap()

    wts = {}
    for name, dims in MLP_DIMS.items():
        for li, (din, dout) in enumerate(zip(dims[:-1], dims[1:])):
            for j, (r0, r1) in enumerate(_w_chunks(din)):
                wts[f"{name}_w{li}_k{j}"] = nc.dram_tensor(
                    f"{name}_w{li}_k{j}", [r1 - r0, dout], f32,
                    kind="ExternalInput").ap()
            wts[f"{name}_b{li}"] = nc.dram_tensor(
                f"{name}_b{li}", [dout, 1], f32, kind="ExternalInput").ap()

    fusedT = nc.dram_tensor("fusedT", [C, NQ], f32, kind="ExternalOutput").ap()
    conf_out = nc.dram_tensor("conf_out", [1, NQ], f32,
                              kind="ExternalOutput").ap()

    with TileContext(nc) as tc:
        with (
            tc.tile_pool(name="inp", bufs=1) as inp,
            tc.tile_pool(name="wp", bufs=1) as wp,
            tc.tile_pool(name="act", bufs=2) as actp,
            tc.tile_pool(name="psum", bufs=2, space="PSUM") as psum,
        ):
            f2s = inp.tile([C, NQ], f32, tag="f2s")
            nc.sync.dma_start(out=f2s[:], in_=f2dT[:])
            f3s = inp.tile([C, NQ], f32, tag="f3s")
            nc.sync.dma_start(out=f3s[:], in_=f3dT[:])
            x2s = inp.tile([C, NQ], f32, tag="x2s")
            x3s = inp.tile([C, NQ], f32, tag="x3s")
            ident = inp.tile([128, 128], f32, tag="ident")
            make_identity(nc, ident[:])
            idxs_all = inp.tile([128, K_OUT * QT], i32, tag="idxs_all")
            nc.sync.dma_start(
                out=idxs_all[:].rearrange("p (t k) -> p t k", k=K_OUT),
                in_=idx16[:, :].rearrange("(t p) k -> p t k", p=128))
            # per query-tile: gather 16 neighbor rows of f2d||f3d, mean,
            # transpose, and add to f2dT/f3dT to form the fusion inputs
            with tc.tile_pool(name="gath", bufs=2) as gathp:
                for t in range(QT):
                    tcols = slice(t * 128, (t + 1) * 128)
                    gt = gathp.tile([128, K_OUT * 2 * C], f32, tag="gt")
                    for j in range(K_OUT):
                        nc.gpsimd.indirect_dma_start(
                            out=gt[:, j * 2 * C:(j + 1) * 2 * C],
                            out_offset=None,
                            in_=f23[:, :],
                            in_offset=bass.IndirectOffsetOnAxis(
                                ap=idxs_all[:, t * K_OUT + j:
                                            t * K_OUT + j + 1], axis=0),
                        )
                    mean = gathp.tile([128, 2 * C], f32, tag="mean")
                    nc.vector.tensor_reduce(
                        out=mean[:],
                        in_=gt[:].rearrange("p (k c) -> p c k", k=K_OUT),
                        axis=mybir.AxisListType.X,
                        op=ALU.add,
                    )
                    nc.vector.tensor_scalar_mul(mean[:], mean[:],
                                                1.0 / K_OUT)
                    for half, dst, base in ((0, x2s, f2s), (1, x3s, f3s)):
                        ptT = psum.tile([C, 128], f32, tag="trp")
                        nc.tensor.transpose(
                            ptT[:C, :], mean[:, half * C:(half + 1) * C],
                            ident[:])
                        nc.vector.tensor_add(dst[:, tcols], base[:, tcols],
                                             ptT[:C, :])
            geos = inp.tile([6, NQ], f32, tag="geos")
            nc.sync.dma_start(out=geos[:], in_=geoT[:])
            mvs = inp.tile([1, NQ], f32, tag="mvs")
            nc.sync.dma_start(out=mvs[:], in_=mv[:])
            scs = inp.tile([1, NQ], f32, tag="scs")
            nc.sync.dma_start(out=scs[:], in_=scv[:])
            ones96 = inp.tile([1, C], f32, tag="ones96")
            nc.vector.memset(ones96[:], 1.0)

            ws = {}
            for key, ap_ in wts.items():
                tile_ = wp.tile(list(ap_.shape), f32, tag=key)
                nc.sync.dma_start(out=tile_[:], in_=ap_[:])
                ws[key] = tile_

            def mlp(name, rhs_list, acts):
                """rhs_list: list of already-sliced (k x 512) APs, in the
                same order as the L0 weight row chunks."""
                dims = MLP_DIMS[name]
                h = None
                for li, (din, dout) in enumerate(zip(dims[:-1], dims[1:])):
                    b = ws[f"{name}_b{li}"]
                    pt = psum.tile([dout, 512], f32, tag="mm")
                    if li == 0:
                        n_in = len(rhs_list)
                        assert n_in == len(_w_chunks(din))
                        for j, src in enumerate(rhs_list):
                            nc.tensor.matmul(
                                pt[:dout, :], ws[f"{name}_w0_k{j}"][:], src,
                                start=(j == 0), stop=(j == n_in - 1),
                            )
                    else:
                        nc.tensor.matmul(pt[:dout, :],
                                         ws[f"{name}_w{li}_k0"][:], h[:],
                                         start=True, stop=True)
                    func = acts[li]
                    hn = actp.tile([dout, 512], f32, tag=f"{name}_h{li}")
                    nc.scalar.activation(hn[:], pt[:dout, :], func,
                                         bias=b[:, 0:1])
                    h = hn
                return h

            for ch in range(NQ // 512):
                cs = slice(ch * 512, (ch + 1) * 512)
                bg = mlp("base_gate", [f2s[:, cs], f3s[:, cs]],
                         [AF.Relu, AF.Relu, AF.Sigmoid])
                sg = mlp("spatial_attn", [x2s[:, cs], x3s[:, cs]],
                         [AF.Relu, AF.Relu, AF.Sigmoid])
                cf = mlp("confidence", [f2s[:, cs], f3s[:, cs]],
                         [AF.Relu, AF.Sigmoid])
                ge = mlp("geo_encoder", [geos[:, cs]],
                         [AF.Relu, AF.Relu])
                gg = mlp("consistency",
                         [f2s[:, cs], f3s[:, cs], ge[:]],
                         [AF.Relu, AF.Relu, AF.Sigmoid])

                # gate_fusion L0: w0 is (3 x 16); feed the three 1-row gates
                # as K=1 accumulating matmuls (no partition-offset writes)
                ptf = psum.tile([16, 512], f32, tag="mm")
                nc.tensor.matmul(ptf[:16, :], ws["gate_fusion_w0_k0"][:],
                                 bg[:], start=True, stop=False)
                nc.tensor.matmul(ptf[:16, :], ws["gate_fusion_w0_k1"][:],
                                 sg[:], start=False, stop=False)
                nc.tensor.matmul(ptf[:16, :], ws["gate_fusion_w0_k2"][:],
                                 gg[:], start=False, stop=True)
                fh = actp.tile([16, 512], f32, tag="gf_h0")
                nc.scalar.activation(fh[:], ptf[:16, :], AF.Relu,
                                     bias=ws["gate_fusion_b0"][:, 0:1])
                ptg = psum.tile([1, 512], f32, tag="mm")
                nc.tensor.matmul(ptg[:1, :], ws["gate_fusion_w1_k0"][:],
                                 fh[:], start=True, stop=True)
                fg = actp.tile([1, 512], f32, tag="gf_h1")
                nc.scalar.activation(fg[:], ptg[:1, :], AF.Sigmoid,
                                     bias=ws["gate_fusion_b1"][:, 0:1])

                # valid-mask scaling of the gate and confidence
                fgs = actp.tile([1, 512], f32, tag="fgs")
                nc.vector.tensor_mul(fgs[:], fg[:], mvs[:, cs])
                cfs = actp.tile([1, 512], f32, tag="cfs")
                nc.vector.tensor_mul(cfs[:], cf[:], scs[:, cs])
                nc.sync.dma_start(out=conf_out[:, cs], in_=cfs[:])

                # broadcast gate over 96 channels and mix
                gb = psum.tile([C, 512], f32, tag="gb")
                nc.tensor.matmul(gb[:], ones96[:], fgs[:],
                                 start=True, stop=True)
                om = actp.tile([C, 512], f32, tag="om")
                nc.scalar.activation(om[:], gb[:], AF.Copy,
                                     bias=1.0, scale=-1.0)
                m1 = actp.tile([C, 512], f32, tag="m1")
                nc.vector.tensor_mul(m1[:], gb[:], f2s[:, cs])
                m2 = actp.tile([C, 512], f32, tag="m2")
                nc.vector.tensor_mul(m2[:], om[:], f3s[:, cs])
                nc.vector.tensor_add(m1[:], m1[:], m2[:])
                nc.sync.dma_start(out=fusedT[:, cs], in_=m1[:])

    nc.compile()
    return nc


_CACHE = {}


def _get_programs():
    if "a" not in _CACHE:
        _CACHE["a"] = build_phase_a()
    if "b" not in _CACHE:
        _CACHE["b"] = build_phase_b()
    return _CACHE["a"], _CACHE["b"]


def _mlp_param_maps(params):
    out = {}
    for name, layers in params.items():
        for li, (w, b) in enumerate(layers):
            w = np.asarray(w, np.float32)
            b = np.asarray(b, np.float32)
            wT = np.ascontiguousarray(w.T)
            for j, (r0, r1) in enumerate(_w_chunks(wT.shape[0])):
                out[f"{name}_w{li}_k{j}"] = np.ascontiguousarray(wT[r0:r1])
            out[f"{name}_b{li}"] = np.ascontiguousarray(b[:, None])
    return out


def kernel(f2d, f3d, xyz, valid_mask, params):
    f2d = np.asarray(f2d, np.float32)
    f3d = np.asarray(f3d, np.float32)
    xyz = np.asarray(xyz, np.float32)
    vm = np.asarray(valid_mask)

    nc_a, nc_b = _get_programs()

    x, y, z = xyz[:, 0], xyz[:, 1], xyz[:, 2]
    sq = x * x
    sq = sq + y * y
    sq = sq + z * z
    ones = np.ones(N, np.float32)
    c5 = np.ascontiguousarray(np.stack([x, y, z, ones, sq]))
    q5_full = np.stack([2.0 * x, 2.0 * y, 2.0 * z, -sq, -ones])
    qpt_full = np.ascontiguousarray(np.stack([x, y, z, sq], axis=1))
    pgt = np.stack([x, y, z, sq], axis=1).reshape(NG, PG_W)
    pgt = np.ascontiguousarray(pgt)
    f23c = np.ascontiguousarray(np.concatenate([f2d, f3d], axis=1))

    in_maps_a = []
    for cid in range(N_CORES):
        qs = slice(cid * NQ, (cid + 1) * NQ)
        in_maps_a.append({
            "c5": c5,
            "q5": np.ascontiguousarray(q5_full[:, qs]),
            "qpt": np.ascontiguousarray(qpt_full[qs]),
            "pg": pgt,
        })
    import time as _time
    t_a0 = _time.time()
    res_a = run_bass_kernel_spmd(nc_a, in_maps_a, list(range(N_CORES)))
    t_a = _time.time() - t_a0

    pos = np.concatenate([res_a.results[c]["pos_out"]
                          for c in range(N_CORES)], axis=0)
    gidx_all = np.concatenate([res_a.results[c]["gidx_out"]
                               for c in range(N_CORES)], axis=0)
    # absolute neighbor index = selected_group * G + offset_within_group
    grp = np.take_along_axis(gidx_all, pos >> 4, axis=1)
    idx = (grp * G + (pos & 15)).astype(np.int32)
    globals()["_last_idx"] = idx
    globals()["_t_phase_a"] = t_a

    # ---- host: normal estimation via eigh on CPU (matches reference impl)
    import jax
    import jax.numpy as jnp
    cpu = jax.devices("cpu")[0]
    idx_nm = idx[:, 1:9]
    with jax.default_device(cpu):
        jxyz = jnp.asarray(xyz)
        nbr = jxyz[jnp.asarray(idx_nm)]
        centered = nbr - jxyz[:, None, :]
        cov = jnp.einsum('nkd,nke->nde', centered, centered)
        _, evecs = jnp.linalg.eigh(cov)
        normals = evecs[:, :, 0]
        normals = normals / jnp.maximum(
            jnp.linalg.norm(normals, axis=1, keepdims=True), 1e-12)
        normals = np.asarray(normals)

    vw = vm.astype(np.float32)
    mvv = vw + np.float32(0.1) * (np.float32(1.0) - vw)
    scvv = vw * np.float32(0.9) + np.float32(0.1)
    geo = np.concatenate([xyz, normals], axis=1)

    wmaps = _mlp_param_maps(params)
    in_maps_b = []
    for cid in range(N_CORES):
        qs = slice(cid * NQ, (cid + 1) * NQ)
        m = {
            "f2dT": np.ascontiguousarray(f2d[qs].T),
            "f3dT": np.ascontiguousarray(f3d[qs].T),
            "f23": f23c,
            "idx16": np.ascontiguousarray(idx[qs]),
            "geoT": np.ascontiguousarray(geo[qs].T),
            "mv": np.ascontiguousarray(mvv[qs][None, :]),
            "scv": np.ascontiguousarray(scvv[qs][None, :]),
        }
        m.update(wmaps)
        in_maps_b.append(m)
    t_b0 = _time.time()
    res_b = run_bass_kernel_spmd(nc_b, in_maps_b, list(range(N_CORES)))
    globals()["_t_phase_b"] = _time.time() - t_b0

    fused = np.concatenate([res_b.results[c]["fusedT"].T
                            for c in range(N_CORES)], axis=0)
    conf = np.concatenate([res_b.results[c]["conf_out"].T
                           for c in range(N_CORES)], axis=0)
    return np.ascontiguousarray(fused), np.ascontiguousarray(conf)
